# revision 11
# baseline (speedup 1.0000x reference)
"""Trainium2 Bass kernel for nn_EventTemplateBank (batched 1-D template-bank conv).

Math: score[b,t,e] = sum_{f,l} delayed[e,f,l] * x[b, t+40-l, f] / (L*F),
with delayed = delay-shifted templates (zero fill) and x zero-padded.

Device formulation (per core, data-parallel over batch):
  Exact-shift feature-packed Toeplitz. Contraction partitions hold
  (feature, tap) pairs with a 21-tap chunk, K = 6*21 = 126. Output columns
  carry Q=21 positions (t = 21n + d). Because chunk stride == column
  stride, chunk k of column n is column n+k of a single NON-EXPANDED
  buffer:
      Xa[(f,c), u] = x[b, 21u + c - 39, f]        (grid a)
      Xb[(f,c), u] = x[b, 21u + c - 23, f]        (grid b, offset +16)
  M-tiles over d: (0..7, 8..15) x 16 events use grid a with 5 chunks;
  (16..20) x 16 events (80 rows) spans only 84 taps, so grid b covers it
  with 4 chunks:
      ps[m][(dd,e), n] += sum_k W[m,k].T @ X[:, n+k]
  14 column-passes per column -> 175k passes/core (72.9 us PE floor) and
  only ~16 MB DMA/core (~50 us): comfortably PE-bound.
  PSUM f32 -> bf16 evac; host upcasts and re-permutes (t = 21n + 8m + dd).
"""

import numpy as np
import ml_dtypes

import concourse.mybir as mybir
from concourse import bacc
from concourse.bass_utils import run_bass_kernel_spmd
from concourse.tile import TileContext

BF16 = ml_dtypes.bfloat16

# Problem shapes (hardcoded per contract)
B, S, F = 64, 32768, 6
E, L = 16, 80
MAX_DELAY = 10

NCORES = 8
BPC = B // NCORES          # batches per core
Q = 21                     # output positions per rhs column (== chunk stride)
C = 21                     # taps per feature per chunk
KP = F * C                 # 126 contraction partitions
NM = 3                     # M-tiles
TILES = [(0, 8, 5), (8, 8, 5), (16, 5, 4)]   # (d0, width, chunks); grid b for last
HALO = 4                   # max chunk shift in columns
NCOLB = (S + Q - 1) // Q   # 1561 output columns per batch
NCX = NCOLB + HALO         # 1565 X columns per batch (per grid)
CTOT = BPC * NCOLB         # 12488 output columns per core
PADL = 39                  # grid a sample = 21u + c - 39
OFFB = 16                  # grid b sample = 21u + c - 23

# Per-batch blocks (blocks may not cross batch boundaries: the X halo
# mapping u = 1565*b + n is per-batch). Batch 0 starts small so the PE
# starts after ~0.25MB and ramps its p-state on cheap blocks.
_B0 = [128, 192, 448, 405, 388]
_BN = [391, 390, 390, 390]
assert sum(_B0) == NCOLB and sum(_BN) == NCOLB
BLOCKS = []                # (batch, offset-in-batch, ncols)
for _bb in range(BPC):
    _o = 0
    for _n in (_B0 if _bb == 0 else _BN):
        BLOCKS.append((_bb, _o, _n))
        _o += _n
NBLK = len(BLOCKS)         # 33
# X DRAM layout: per block, [a-seg n+4 | b-seg n+4] contiguous per partition
XSEG = [2 * (n + HALO) for (_, _, n) in BLOCKS]
XOFF = [sum(XSEG[:i]) for i in range(NBLK)]
XTOT = sum(XSEG)
# W layout: [m0: 5*128 | m1: 5*128 | m2: 4*80] = 1600 columns
WOFF = [0, 5 * 128, 10 * 128]
WTOT = 10 * 128 + 4 * 80

LAST_RESULT = None         # BassKernelResults of the most recent run (for profiling)


def _build_weights(templates: np.ndarray, onset_delays: np.ndarray) -> np.ndarray:
    d = np.round(np.clip(onset_delays, -MAX_DELAY, MAX_DELAY)).astype(np.int64)
    idx = np.arange(L)
    src = idx[None, None, :] - d[:, :, None]                 # (E,F,L)
    valid = (src >= 0) & (src < L)
    delayed = np.take_along_axis(templates, np.clip(src, 0, L - 1), axis=2)
    delayed = np.where(valid, delayed, 0.0).astype(np.float32) / float(L * F)

    W = np.zeros((KP, WTOT), dtype=np.float32)
    for mi, (d0, wdt, nch) in enumerate(TILES):
        base = WOFF[mi]
        for f in range(F):
            for c in range(C):
                for k in range(nch):
                    for dd in range(wdt):
                        if nch == 5:
                            l = (d0 + dd) + 79 - 21 * k - c      # grid a
                        else:
                            l = (d0 + dd) + 63 - 21 * k - c      # grid b
                        if 0 <= l < L:
                            col = base + k * wdt * E + dd * E
                            W[f * C + c, col:col + E] = delayed[:, f, l]
    return np.ascontiguousarray(W).astype(BF16)


def _build_xsc(x: np.ndarray) -> np.ndarray:
    need = Q * (NCX - 1) + C + OFFB
    xpad = np.zeros((B, PADL + need, F), dtype=np.float32)
    xpad[:, PADL:PADL + S, :] = x
    sb, st, sf = xpad.strides
    Va = np.lib.stride_tricks.as_strided(
        xpad, shape=(B, F, C, NCX), strides=(sb, sf, st, Q * st))
    Vb = np.lib.stride_tricks.as_strided(
        xpad[:, OFFB:], shape=(B, F, C, NCX), strides=(sb, sf, st, Q * st))
    Xa = Va.astype(BF16).reshape(B, KP, NCX)
    Xb = Vb.astype(BF16).reshape(B, KP, NCX)
    out = np.empty((NCORES, KP, XTOT), dtype=BF16)
    for core in range(NCORES):
        for blk, (bb, o, n) in enumerate(BLOCKS):
            b = core * BPC + bb
            seg = XOFF[blk]
            out[core, :, seg:seg + n + HALO] = Xa[b][:, o:o + n + HALO]
            out[core, :, seg + n + HALO:seg + 2 * (n + HALO)] = Xb[b][:, o:o + n + HALO]
    return np.ascontiguousarray(out)


def _build_program():
    f32 = mybir.dt.float32
    bf16 = mybir.dt.bfloat16
    nc = bacc.Bacc("TRN2", target_bir_lowering=False, debug=False)
    xsc = nc.dram_tensor("xsc", [KP, XTOT], bf16, kind="ExternalInput")
    w = nc.dram_tensor("w", [KP, WTOT], bf16, kind="ExternalInput")
    osc = nc.dram_tensor("osc", [128, NM * CTOT], bf16, kind="ExternalOutput")

    with TileContext(nc) as tc:
        with (
            tc.tile_pool(name="wp", bufs=1) as wp,
            tc.tile_pool(name="xp", bufs=6) as xp,
            tc.tile_pool(name="pp", bufs=8, space="PSUM") as pp,
            tc.tile_pool(name="op", bufs=6) as op,
        ):
            wt = wp.tile([KP, WTOT], bf16)
            xtiles = {}

            def issue_w(mi):
                lo = WOFF[mi]
                hi = WOFF[mi + 1] if mi + 1 < NM else WTOT
                nc.sync.dma_start(out=wt[:, lo:hi], in_=w[:, lo:hi])

            def issue_x(blk):
                seg, width = XOFF[blk], XSEG[blk]
                xt = xp.tile([KP, width], bf16, tag="xt", name=f"xt_{blk}")
                nc.sync.dma_start(out=xt, in_=xsc[:, seg:seg + width])
                xtiles[blk] = xt

            def wslice(mi, k):
                wdt = TILES[mi][1]
                lo = WOFF[mi] + k * wdt * E
                return wt[:, lo:lo + wdt * E]

            # DMA order: first matmul gates on W(m0)+X0 (~0.25MB).
            issue_w(0); issue_x(0)
            issue_w(1); issue_x(1)
            issue_w(2); issue_x(2)

            for blk, (bb, o, n) in enumerate(BLOCKS):
                if blk + 3 < NBLK:
                    issue_x(blk + 3)
                xt = xtiles.pop(blk)
                goff = bb * NCOLB + o
                ot = op.tile([128, NM * n], bf16, tag="ot", name=f"ot_{blk}")
                for mi, (d0, wdt, nch) in enumerate(TILES):
                    rows = wdt * E
                    ps = pp.tile([rows, n], f32, tag="ps", name=f"ps_{blk}_{mi}")
                    xbase = 0 if nch == 5 else n + HALO
                    for k in range(nch):
                        nc.tensor.matmul(
                            ps,
                            wslice(mi, k),
                            xt[:, xbase + k:xbase + k + n],
                            start=(k == 0),
                            stop=(k == nch - 1),
                        )
                    # rows 80:128 of the m2 segment are never written; the
                    # block DMA below ships them as don't-care filler.
                    nc.vector.tensor_copy(out=ot[0:rows, mi * n:(mi + 1) * n], in_=ps)
                nc.sync.dma_start(
                    out=osc[:, NM * goff:NM * (goff + n)], in_=ot
                )
    nc.compile()   # bacc passes: split multi-waits (HW allows 1 wait/inst), DCE, reg alloc
    return nc


def kernel(x: np.ndarray, templates: np.ndarray, onset_delays: np.ndarray) -> np.ndarray:
    global LAST_RESULT
    x = np.ascontiguousarray(x, dtype=np.float32)
    templates = np.asarray(templates, dtype=np.float32)
    onset_delays = np.asarray(onset_delays, dtype=np.float32)

    W = _build_weights(templates, onset_delays)
    Xsc = _build_xsc(x)                                   # (NCORES, KP, XTOT)

    nc = _build_program()
    in_maps = [{"xsc": Xsc[c], "w": W} for c in range(NCORES)]
    res = run_bass_kernel_spmd(nc, in_maps, core_ids=list(range(NCORES)))
    LAST_RESULT = res

    osc = np.stack([r["osc"] for r in res.results], axis=0)   # (NCORES,128,NM*CTOT)
    osc = osc.astype(np.float32)
    O = np.empty((NCORES, 128, NM, CTOT), dtype=np.float32)   # block segs are [NM, n]
    for (bb, o, n) in BLOCKS:
        goff = bb * NCOLB + o
        O[:, :, :, goff:goff + n] = (
            osc[:, :, NM * goff:NM * (goff + n)].reshape(NCORES, 128, NM, n)
        )
    # assemble d = 8m + dd (m2 valid rows: dd < 5)
    Od = O.reshape(NCORES, 8, E, NM, BPC, NCOLB)          # c, dd, e, m, b, n
    acc = np.empty((NCORES, BPC, NCOLB, Q, E), dtype=np.float32)
    for mi, (d0, wdt, nch) in enumerate(TILES):
        acc[:, :, :, d0:d0 + wdt, :] = (
            Od[:, :wdt, :, mi].transpose(0, 3, 4, 1, 2)
        )
    o = acc.reshape(B, NCOLB * Q, E)[:, :S, :]
    o = np.ascontiguousarray(o)
    o[:, S - 1, :] = 0.0                                   # reference zero-pads last column
    return o


# revision 15
# speedup vs baseline: 1.0188x; 1.0188x over previous
"""Trainium2 Bass kernel for nn_EventTemplateBank (batched 1-D template-bank conv).

Math: score[b,t,e] = sum_{f,l} delayed[e,f,l] * x[b, t+40-l, f] / (L*F),
with delayed = delay-shifted templates (zero fill) and x zero-padded.

Device formulation (per core, data-parallel over batch):
  Feature-packed Toeplitz: contraction partitions hold (feature, tap-chunk)
  pairs, K = 6*21 = 126, so one matmul contracts all 6 features over a
  21-tap window. Each rhs column covers Q=24 output positions; the 103-tap
  span (24+79) is accumulated over NCH=5 chunks. Outputs (d in [0,24),
  e in [0,16)) form NM=3 M-tiles of 128.
    X[(f,c), j, col=n] = x[b, 24n + 21j + c - 39, f]      (bf16, host im2col)
    W[(f,c), j, m, (dd,e)] = delayed[e, f, 8m+dd+79-21j-c] / 480
    ps[m][(dd,e), n] += sum_j W[:, j, m].T @ X[:, j, n-block]
  163,920 column-passes/core (vs 196,704 for the single-feature window) =
  68.3 us PE floor; 22.6 MB DMA/core ~= 71 us: balanced rooflines.
  PSUM f32 -> bf16 evac; host upcasts and re-permutes (t = 24n + 8m + dd).
"""

import numpy as np
import ml_dtypes

import concourse.mybir as mybir
from concourse import bacc
from concourse.bass_utils import run_bass_kernel_spmd
from concourse.tile import TileContext

BF16 = ml_dtypes.bfloat16

# Problem shapes (hardcoded per contract)
B, S, F = 64, 32768, 6
E, L = 16, 80
MAX_DELAY = 10

NCORES = 8
BPC = B // NCORES          # batches per core
Q = 24                     # output positions per rhs column
C = 21                     # taps per feature per chunk
NCH = 5                    # accumulation chunks (cover 24+79 = 103 <= 105 taps)
KP = F * C                 # 126 contraction partitions
NM = 3                     # M-tiles: (dd in [0,8)) x (e in [0,16)) per tile
PADL = 39                  # chunk sample index = 24n + 21j + c - 39
NCOLB = (S + Q - 1) // Q   # 1366 columns per batch
CTOT = BPC * NCOLB         # 10928 columns per core
# Small blocks first so the PE starts after ~0.15MB of input and ramps its
# p-state on cheap blocks; tiny trailing blocks keep the final
# matmul->cast->store drain chain short.
BLOCKS = [64, 96, 160, 256, 384] + [512] * 19 + [144, 96]
assert sum(BLOCKS) == CTOT
NBLK = len(BLOCKS)
BOFF = [sum(BLOCKS[:i]) for i in range(NBLK)]
N_JOUTER = 5               # leading blocks run j-outer (gate on per-j W pieces)

LAST_RESULT = None         # BassKernelResults of the most recent run (for profiling)


def _build_weights(templates: np.ndarray, onset_delays: np.ndarray) -> np.ndarray:
    """W[(f,c), j, m, 16dd+e] = delayed[e, f, 8m+dd+79-21j-c] / (L*F)."""
    d = np.round(np.clip(onset_delays, -MAX_DELAY, MAX_DELAY)).astype(np.int64)
    idx = np.arange(L)
    src = idx[None, None, :] - d[:, :, None]                 # (E,F,L)
    valid = (src >= 0) & (src < L)
    delayed = np.take_along_axis(templates, np.clip(src, 0, L - 1), axis=2)
    delayed = np.where(valid, delayed, 0.0).astype(np.float32) / float(L * F)

    f_i = np.arange(F)[:, None, None, None, None]
    c_i = np.arange(C)[None, :, None, None, None]
    j_i = np.arange(NCH)[None, None, :, None, None]
    dd_i = np.arange(8)[None, None, None, :, None]
    m_i = np.arange(NM)[None, None, None, None, :]
    l = (8 * m_i + dd_i) + 79 - 21 * j_i - c_i               # (F,C,NCH,8,NM)
    ok = (l >= 0) & (l < L)
    g = delayed[:, f_i, np.clip(l, 0, L - 1)]                # (E,F,C,NCH,8,NM)
    g = np.where(ok[None], g, 0.0)
    # -> [(f,c), j, m, dd, e]
    W = g.transpose(1, 2, 3, 5, 4, 0).reshape(KP, NCH, NM, 8 * E)
    return np.ascontiguousarray(W).astype(BF16)


def _build_xsc(x: np.ndarray) -> np.ndarray:
    """Xsc[core, (f,c), :] = block-major concat of [NCH, n_blk] chunk rows:
    chunk j of column col = 1366*b_local + n reads x[b, 24n + 21j + c - 39, f]."""
    need = Q * (NCOLB - 1) + 21 * (NCH - 1) + C
    xpad = np.zeros((B, PADL + need, F), dtype=np.float32)
    xpad[:, PADL:PADL + S, :] = x
    sb, st, sf = xpad.strides
    # V[b, (f,c), j, n] = xpad[b, 24n + 21j + c, f]
    V = np.lib.stride_tricks.as_strided(
        xpad, shape=(B, F, C, NCH, NCOLB), strides=(sb, sf, st, 21 * st, Q * st)
    )
    V16 = V.astype(BF16).reshape(B, KP, NCH, NCOLB)
    Xc = np.empty((NCORES, KP, NCH, CTOT), dtype=BF16)
    for b in range(B):
        core, i = divmod(b, BPC)
        Xc[core, :, :, i * NCOLB:(i + 1) * NCOLB] = V16[b]
    out = np.empty((NCORES, KP, NCH * CTOT), dtype=BF16)
    for off, n in zip(BOFF, BLOCKS):
        out[:, :, NCH * off:NCH * (off + n)] = (
            Xc[:, :, :, off:off + n].reshape(NCORES, KP, NCH * n)
        )
    return np.ascontiguousarray(out)


def _build_program():
    f32 = mybir.dt.float32
    bf16 = mybir.dt.bfloat16
    nc = bacc.Bacc("TRN2", target_bir_lowering=False, debug=False)
    xsc = nc.dram_tensor("xsc", [KP, NCH * CTOT], bf16, kind="ExternalInput")
    w = nc.dram_tensor("w", [KP, NCH, NM, 128], bf16, kind="ExternalInput")
    osc = nc.dram_tensor("osc", [128, NM * CTOT], bf16, kind="ExternalOutput")

    with TileContext(nc) as tc:
        with (
            tc.tile_pool(name="wp", bufs=1) as wp,
            tc.tile_pool(name="xp", bufs=6) as xp,
            tc.tile_pool(name="pp", bufs=8, space="PSUM") as pp,
            tc.tile_pool(name="op", bufs=6) as op,
        ):
            wt = wp.tile([KP, NCH * NM * 128], bf16)     # [(f,c), (j, m, col)]
            wr = w.rearrange("k j m n -> k (j m n)")
            xtiles = {}

            def issue_w(j, m=None):
                if m is None:
                    sl = slice(j * NM * 128, (j + 1) * NM * 128)
                else:
                    sl = slice((j * NM + m) * 128, (j * NM + m + 1) * 128)
                nc.sync.dma_start(out=wt[:, sl], in_=wr[:, sl])

            def issue_x(blk):
                off, n = BOFF[blk], BLOCKS[blk]
                xt = xp.tile([KP, NCH * n], bf16, tag="xt", name=f"xt_{blk}")
                nc.sync.dma_start(out=xt, in_=xsc[:, NCH * off:NCH * (off + n)])
                xtiles[blk] = xt

            def wslice(j, m):
                return wt[:, (j * NM + m) * 128:(j * NM + m + 1) * 128]

            # DMA order: first matmul gates on W(j0,m0)+X0 (~0.11MB); later
            # pieces and x blocks stream in behind it.
            issue_w(0, 0); issue_x(0)
            issue_w(0, 1); issue_w(0, 2)
            issue_x(1)
            issue_w(1); issue_x(2)
            issue_w(2); issue_w(3); issue_w(4)

            for blk in range(NBLK):
                off, n = BOFF[blk], BLOCKS[blk]
                if blk + 3 < NBLK:
                    issue_x(blk + 3)
                xt = xtiles.pop(blk)
                pss = [
                    pp.tile([128, n], f32, tag="ps", name=f"ps_{blk}_{m}")
                    for m in range(NM)
                ]
                ot = op.tile([128, NM * n], bf16, tag="ot", name=f"ot_{blk}")

                def evac(m, n=n, pss=pss, ot=ot):
                    nc.vector.tensor_copy(out=ot[:, m * n:(m + 1) * n], in_=pss[m])

                if blk < N_JOUTER:
                    # j-outer: each arriving W(j) piece feeds all 3 M-tiles.
                    for j in range(NCH):
                        for m in range(NM):
                            nc.tensor.matmul(
                                pss[m],
                                wslice(j, m),
                                xt[:, j * n:(j + 1) * n],
                                start=(j == 0),
                                stop=(j == NCH - 1),
                                skip_group_check=True,
                            )
                    for m in range(NM):
                        evac(m)
                else:
                    # m-outer: M-tiles complete one after another, so PSUM
                    # evacuation staggers across the block.
                    for m in range(NM):
                        for j in range(NCH):
                            nc.tensor.matmul(
                                pss[m],
                                wslice(j, m),
                                xt[:, j * n:(j + 1) * n],
                                start=(j == 0),
                                stop=(j == NCH - 1),
                            )
                        evac(m)
                nc.sync.dma_start(
                    out=osc[:, NM * off:NM * (off + n)], in_=ot
                )
    nc.compile()   # bacc passes: split multi-waits (HW allows 1 wait/inst), DCE, reg alloc
    return nc


def kernel(x: np.ndarray, templates: np.ndarray, onset_delays: np.ndarray) -> np.ndarray:
    global LAST_RESULT
    x = np.ascontiguousarray(x, dtype=np.float32)
    templates = np.asarray(templates, dtype=np.float32)
    onset_delays = np.asarray(onset_delays, dtype=np.float32)

    W = _build_weights(templates, onset_delays)
    Xsc = _build_xsc(x)                                   # (NCORES, KP, NCH*CTOT)

    nc = _build_program()
    in_maps = [{"xsc": Xsc[c], "w": W} for c in range(NCORES)]
    res = run_bass_kernel_spmd(nc, in_maps, core_ids=list(range(NCORES)))
    LAST_RESULT = res

    osc = np.stack([r["osc"] for r in res.results], axis=0)   # (NCORES,128,NM*CTOT)
    osc = osc.astype(np.float32)
    O = np.empty((NCORES, 128, NM, CTOT), dtype=np.float32)
    for off, n in zip(BOFF, BLOCKS):
        O[:, :, :, off:off + n] = (
            osc[:, :, NM * off:NM * (off + n)].reshape(NCORES, 128, NM, n)
        )
    o = O.reshape(NCORES, 8, E, NM, BPC, NCOLB)           # c, dd, e, m, b, n
    o = o.transpose(0, 4, 5, 3, 1, 2)                      # c, b, n, m, dd, e
    o = np.ascontiguousarray(o).reshape(B, NCOLB * Q, E)[:, :S, :]
    o = np.ascontiguousarray(o)
    o[:, S - 1, :] = 0.0                                   # reference zero-pads last column
    return o


# revision 16
# speedup vs baseline: 1.0255x; 1.0066x over previous
"""Trainium2 Bass kernel for nn_EventTemplateBank (batched 1-D template-bank conv).

Math: score[b,t,e] = sum_{f,l} delayed[e,f,l] * x[b, t+40-l, f] / (L*F),
with delayed = delay-shifted templates (zero fill) and x zero-padded.

Device formulation (per core, data-parallel over batch):
  Feature-packed Toeplitz: contraction partitions hold (feature, tap-chunk)
  pairs, K = 6*21 = 126, so one matmul contracts all 6 features over a
  21-tap window. Each rhs column covers Q=24 output positions; the 103-tap
  span (24+79) is accumulated over NCH=5 chunks. Outputs (d in [0,24),
  e in [0,16)) form NM=3 M-tiles of 128.
    X[(f,c), j, col=n] = x[b, 24n + 21j + c - 39, f]      (bf16, host im2col)
    W[(f,c), j, m, (dd,e)] = delayed[e, f, 8m+dd+79-21j-c] / 480
    ps[m][(dd,e), n] += sum_j W[:, j, m].T @ X[:, j, n-block]
  163,920 column-passes/core (vs 196,704 for the single-feature window) =
  68.3 us PE floor; 22.6 MB DMA/core ~= 71 us: balanced rooflines.
  PSUM f32 -> bf16 evac; host upcasts and re-permutes (t = 24n + 8m + dd).
"""

import numpy as np
import ml_dtypes

import concourse.mybir as mybir
from concourse import bacc
from concourse.bass_utils import run_bass_kernel_spmd
from concourse.tile import TileContext

BF16 = ml_dtypes.bfloat16

# Problem shapes (hardcoded per contract)
B, S, F = 64, 32768, 6
E, L = 16, 80
MAX_DELAY = 10

NCORES = 8
BPC = B // NCORES          # batches per core
Q = 24                     # output positions per rhs column
C = 21                     # taps per feature per chunk
NCH = 5                    # accumulation chunks (cover 24+79 = 103 <= 105 taps)
KP = F * C                 # 126 contraction partitions
NM = 3                     # M-tiles: (dd in [0,8)) x (e in [0,16)) per tile
PADL = 39                  # chunk sample index = 24n + 21j + c - 39
NCOLB = (S + Q - 1) // Q   # 1366 columns per batch
CTOT = BPC * NCOLB         # 10928 columns per core
# Small blocks first so the PE starts after ~0.15MB of input and ramps its
# p-state on cheap blocks; tiny trailing blocks keep the final
# matmul->cast->store drain chain short.
BLOCKS = [128, 128, 256] + [512] * 20 + [176]
assert sum(BLOCKS) == CTOT
NBLK = len(BLOCKS)
BOFF = [sum(BLOCKS[:i]) for i in range(NBLK)]
N_JOUTER = 3               # leading blocks run j-outer (gate on per-j W pieces)

LAST_RESULT = None         # BassKernelResults of the most recent run (for profiling)


def _build_weights(templates: np.ndarray, onset_delays: np.ndarray) -> np.ndarray:
    """W[(f,c), j, m, 16dd+e] = delayed[e, f, 8m+dd+79-21j-c] / (L*F)."""
    d = np.round(np.clip(onset_delays, -MAX_DELAY, MAX_DELAY)).astype(np.int64)
    idx = np.arange(L)
    src = idx[None, None, :] - d[:, :, None]                 # (E,F,L)
    valid = (src >= 0) & (src < L)
    delayed = np.take_along_axis(templates, np.clip(src, 0, L - 1), axis=2)
    delayed = np.where(valid, delayed, 0.0).astype(np.float32) / float(L * F)

    f_i = np.arange(F)[:, None, None, None, None]
    c_i = np.arange(C)[None, :, None, None, None]
    j_i = np.arange(NCH)[None, None, :, None, None]
    dd_i = np.arange(8)[None, None, None, :, None]
    m_i = np.arange(NM)[None, None, None, None, :]
    l = (8 * m_i + dd_i) + 79 - 21 * j_i - c_i               # (F,C,NCH,8,NM)
    ok = (l >= 0) & (l < L)
    g = delayed[:, f_i, np.clip(l, 0, L - 1)]                # (E,F,C,NCH,8,NM)
    g = np.where(ok[None], g, 0.0)
    # -> [(f,c), j, m, dd, e]
    W = g.transpose(1, 2, 3, 5, 4, 0).reshape(KP, NCH, NM, 8 * E)
    return np.ascontiguousarray(W).astype(BF16)


def _build_xsc(x: np.ndarray) -> np.ndarray:
    """Xsc[core, (f,c), :] = block-major concat of [NCH, n_blk] chunk rows:
    chunk j of column col = 1366*b_local + n reads x[b, 24n + 21j + c - 39, f]."""
    need = Q * (NCOLB - 1) + 21 * (NCH - 1) + C
    xpad = np.zeros((B, PADL + need, F), dtype=np.float32)
    xpad[:, PADL:PADL + S, :] = x
    sb, st, sf = xpad.strides
    # V[b, (f,c), j, n] = xpad[b, 24n + 21j + c, f]
    V = np.lib.stride_tricks.as_strided(
        xpad, shape=(B, F, C, NCH, NCOLB), strides=(sb, sf, st, 21 * st, Q * st)
    )
    V16 = V.astype(BF16).reshape(B, KP, NCH, NCOLB)
    Xc = np.empty((NCORES, KP, NCH, CTOT), dtype=BF16)
    for b in range(B):
        core, i = divmod(b, BPC)
        Xc[core, :, :, i * NCOLB:(i + 1) * NCOLB] = V16[b]
    out = np.empty((NCORES, KP, NCH * CTOT), dtype=BF16)
    for off, n in zip(BOFF, BLOCKS):
        out[:, :, NCH * off:NCH * (off + n)] = (
            Xc[:, :, :, off:off + n].reshape(NCORES, KP, NCH * n)
        )
    return np.ascontiguousarray(out)


def _build_program():
    f32 = mybir.dt.float32
    bf16 = mybir.dt.bfloat16
    nc = bacc.Bacc("TRN2", target_bir_lowering=False, debug=False)
    xsc = nc.dram_tensor("xsc", [KP, NCH * CTOT], bf16, kind="ExternalInput")
    w = nc.dram_tensor("w", [KP, NCH, NM, 128], bf16, kind="ExternalInput")
    osc = nc.dram_tensor("osc", [128, NM * CTOT], bf16, kind="ExternalOutput")

    with TileContext(nc) as tc:
        with (
            tc.tile_pool(name="wp", bufs=1) as wp,
            tc.tile_pool(name="xp", bufs=6) as xp,
            tc.tile_pool(name="pp", bufs=8, space="PSUM") as pp,
            tc.tile_pool(name="op", bufs=6) as op,
        ):
            wt = wp.tile([KP, NCH * NM * 128], bf16)     # [(f,c), (j, m, col)]
            wr = w.rearrange("k j m n -> k (j m n)")
            xtiles = {}

            def issue_w(j, m=None):
                if m is None:
                    sl = slice(j * NM * 128, (j + 1) * NM * 128)
                else:
                    sl = slice((j * NM + m) * 128, (j * NM + m + 1) * 128)
                nc.sync.dma_start(out=wt[:, sl], in_=wr[:, sl])

            def issue_x(blk):
                off, n = BOFF[blk], BLOCKS[blk]
                xt = xp.tile([KP, NCH * n], bf16, tag="xt", name=f"xt_{blk}")
                nc.sync.dma_start(out=xt, in_=xsc[:, NCH * off:NCH * (off + n)])
                xtiles[blk] = xt

            def wslice(j, m):
                return wt[:, (j * NM + m) * 128:(j * NM + m + 1) * 128]

            # DMA order: first matmul gates on W(j0,m0)+X0 (~0.11MB); later
            # pieces and x blocks stream in behind it.
            issue_w(0, 0); issue_x(0)
            issue_w(0, 1); issue_w(0, 2)
            issue_x(1)
            issue_w(1); issue_x(2)
            issue_w(2); issue_w(3); issue_w(4)

            for blk in range(NBLK):
                off, n = BOFF[blk], BLOCKS[blk]
                if blk + 3 < NBLK:
                    issue_x(blk + 3)
                xt = xtiles.pop(blk)
                pss = [
                    pp.tile([128, n], f32, tag="ps", name=f"ps_{blk}_{m}")
                    for m in range(NM)
                ]
                ot = op.tile([128, NM * n], bf16, tag="ot", name=f"ot_{blk}")

                def evac(m, n=n, pss=pss, ot=ot):
                    nc.vector.tensor_copy(out=ot[:, m * n:(m + 1) * n], in_=pss[m])

                if blk < N_JOUTER:
                    # j-outer: each arriving W(j) piece feeds all 3 M-tiles.
                    for j in range(NCH):
                        for m in range(NM):
                            nc.tensor.matmul(
                                pss[m],
                                wslice(j, m),
                                xt[:, j * n:(j + 1) * n],
                                start=(j == 0),
                                stop=(j == NCH - 1),
                                skip_group_check=True,
                            )
                    for m in range(NM):
                        evac(m)
                else:
                    # m-outer: M-tiles complete one after another, so PSUM
                    # evacuation staggers across the block.
                    for m in range(NM):
                        for j in range(NCH):
                            nc.tensor.matmul(
                                pss[m],
                                wslice(j, m),
                                xt[:, j * n:(j + 1) * n],
                                start=(j == 0),
                                stop=(j == NCH - 1),
                            )
                        evac(m)
                nc.sync.dma_start(
                    out=osc[:, NM * off:NM * (off + n)], in_=ot
                )
    nc.compile()   # bacc passes: split multi-waits (HW allows 1 wait/inst), DCE, reg alloc
    return nc


def kernel(x: np.ndarray, templates: np.ndarray, onset_delays: np.ndarray) -> np.ndarray:
    global LAST_RESULT
    x = np.ascontiguousarray(x, dtype=np.float32)
    templates = np.asarray(templates, dtype=np.float32)
    onset_delays = np.asarray(onset_delays, dtype=np.float32)

    W = _build_weights(templates, onset_delays)
    Xsc = _build_xsc(x)                                   # (NCORES, KP, NCH*CTOT)

    nc = _build_program()
    in_maps = [{"xsc": Xsc[c], "w": W} for c in range(NCORES)]
    res = run_bass_kernel_spmd(nc, in_maps, core_ids=list(range(NCORES)))
    LAST_RESULT = res

    osc = np.stack([r["osc"] for r in res.results], axis=0)   # (NCORES,128,NM*CTOT)
    osc = osc.astype(np.float32)
    O = np.empty((NCORES, 128, NM, CTOT), dtype=np.float32)
    for off, n in zip(BOFF, BLOCKS):
        O[:, :, :, off:off + n] = (
            osc[:, :, NM * off:NM * (off + n)].reshape(NCORES, 128, NM, n)
        )
    o = O.reshape(NCORES, 8, E, NM, BPC, NCOLB)           # c, dd, e, m, b, n
    o = o.transpose(0, 4, 5, 3, 1, 2)                      # c, b, n, m, dd, e
    o = np.ascontiguousarray(o).reshape(B, NCOLB * Q, E)[:, :S, :]
    o = np.ascontiguousarray(o)
    o[:, S - 1, :] = 0.0                                   # reference zero-pads last column
    return o


# revision 17
# speedup vs baseline: 1.0305x; 1.0049x over previous
"""Trainium2 Bass kernel for nn_EventTemplateBank (batched 1-D template-bank conv).

Math: score[b,t,e] = sum_{f,l} delayed[e,f,l] * x[b, t+40-l, f] / (L*F),
with delayed = delay-shifted templates (zero fill) and x zero-padded.

Device formulation (per core, data-parallel over batch):
  Feature-packed Toeplitz: contraction partitions hold (feature, tap-chunk)
  pairs, K = 6*21 = 126, so one matmul contracts all 6 features over a
  21-tap window. Each rhs column covers Q=24 output positions; the 103-tap
  span (24+79) is accumulated over NCH=5 chunks. Outputs (d in [0,24),
  e in [0,16)) form NM=3 M-tiles of 128.
    X[(f,c), j, col=n] = x[b, 24n + 21j + c - 39, f]      (bf16, host im2col)
    W[(f,c), j, m, (dd,e)] = delayed[e, f, 8m+dd+79-21j-c] / 480
    ps[m][(dd,e), n] += sum_j W[:, j, m].T @ X[:, j, n-block]
  163,920 column-passes/core (vs 196,704 for the single-feature window) =
  68.3 us PE floor; 22.6 MB DMA/core ~= 71 us: balanced rooflines.
  PSUM f32 -> bf16 evac; host upcasts and re-permutes (t = 24n + 8m + dd).
"""

import numpy as np
import ml_dtypes

import concourse.mybir as mybir
from concourse import bacc
from concourse.bass_utils import run_bass_kernel_spmd
from concourse.tile import TileContext

BF16 = ml_dtypes.bfloat16

# Problem shapes (hardcoded per contract)
B, S, F = 64, 32768, 6
E, L = 16, 80
MAX_DELAY = 10

NCORES = 8
BPC = B // NCORES          # batches per core
Q = 24                     # output positions per rhs column
C = 21                     # taps per feature per chunk
NCH = 5                    # accumulation chunks (cover 24+79 = 103 <= 105 taps)
KP = F * C                 # 126 contraction partitions
NM = 3                     # M-tiles: (dd in [0,8)) x (e in [0,16)) per tile
PADL = 39                  # chunk sample index = 24n + 21j + c - 39
NCOLB = (S + Q - 1) // Q   # 1366 columns per batch
CTOT = BPC * NCOLB         # 10928 columns per core
# Small blocks first so the PE starts after ~0.15MB of input and ramps its
# p-state on cheap blocks; tiny trailing blocks keep the final
# matmul->cast->store drain chain short.
BLOCKS = [128, 128, 256] + [512] * 20 + [176]
assert sum(BLOCKS) == CTOT
NBLK = len(BLOCKS)
BOFF = [sum(BLOCKS[:i]) for i in range(NBLK)]
N_JOUTER = 3               # leading blocks run j-outer (gate on per-j W pieces)

LAST_RESULT = None         # BassKernelResults of the most recent run (for profiling)


def _build_weights(templates: np.ndarray, onset_delays: np.ndarray) -> np.ndarray:
    """W[(f,c), j, m, 16dd+e] = delayed[e, f, 8m+dd+79-21j-c] / (L*F)."""
    d = np.round(np.clip(onset_delays, -MAX_DELAY, MAX_DELAY)).astype(np.int64)
    idx = np.arange(L)
    src = idx[None, None, :] - d[:, :, None]                 # (E,F,L)
    valid = (src >= 0) & (src < L)
    delayed = np.take_along_axis(templates, np.clip(src, 0, L - 1), axis=2)
    delayed = np.where(valid, delayed, 0.0).astype(np.float32) / float(L * F)

    f_i = np.arange(F)[:, None, None, None, None]
    c_i = np.arange(C)[None, :, None, None, None]
    j_i = np.arange(NCH)[None, None, :, None, None]
    dd_i = np.arange(8)[None, None, None, :, None]
    m_i = np.arange(NM)[None, None, None, None, :]
    l = (8 * m_i + dd_i) + 79 - 21 * j_i - c_i               # (F,C,NCH,8,NM)
    ok = (l >= 0) & (l < L)
    g = delayed[:, f_i, np.clip(l, 0, L - 1)]                # (E,F,C,NCH,8,NM)
    g = np.where(ok[None], g, 0.0)
    # -> [(f,c), j, m, dd, e]
    W = g.transpose(1, 2, 3, 5, 4, 0).reshape(KP, NCH, NM, 8 * E)
    return np.ascontiguousarray(W).astype(BF16)


def _build_xsc(x: np.ndarray) -> np.ndarray:
    """Xsc[core, (f,c), :] = block-major concat of [NCH, n_blk] chunk rows:
    chunk j of column col = 1366*b_local + n reads x[b, 24n + 21j + c - 39, f]."""
    need = Q * (NCOLB - 1) + 21 * (NCH - 1) + C
    xpad = np.zeros((B, PADL + need, F), dtype=np.float32)
    xpad[:, PADL:PADL + S, :] = x
    sb, st, sf = xpad.strides
    # V[b, (f,c), j, n] = xpad[b, 24n + 21j + c, f]
    V = np.lib.stride_tricks.as_strided(
        xpad, shape=(B, F, C, NCH, NCOLB), strides=(sb, sf, st, 21 * st, Q * st)
    )
    V16 = V.astype(BF16).reshape(B, KP, NCH, NCOLB)
    Xc = np.empty((NCORES, KP, NCH, CTOT), dtype=BF16)
    for b in range(B):
        core, i = divmod(b, BPC)
        Xc[core, :, :, i * NCOLB:(i + 1) * NCOLB] = V16[b]
    out = np.empty((NCORES, KP, NCH * CTOT), dtype=BF16)
    for off, n in zip(BOFF, BLOCKS):
        out[:, :, NCH * off:NCH * (off + n)] = (
            Xc[:, :, :, off:off + n].reshape(NCORES, KP, NCH * n)
        )
    return np.ascontiguousarray(out)


def _build_program():
    f32 = mybir.dt.float32
    bf16 = mybir.dt.bfloat16
    nc = bacc.Bacc("TRN2", target_bir_lowering=False, debug=False)
    xsc = nc.dram_tensor("xsc", [KP, NCH * CTOT], bf16, kind="ExternalInput")
    w = nc.dram_tensor("w", [KP, NCH, NM, 128], bf16, kind="ExternalInput")
    osc = nc.dram_tensor("osc", [128, NM * CTOT], bf16, kind="ExternalOutput")

    with TileContext(nc) as tc:
        with (
            tc.tile_pool(name="wp", bufs=1) as wp,
            tc.tile_pool(name="xp", bufs=6) as xp,
            tc.tile_pool(name="pp", bufs=8, space="PSUM") as pp,
            tc.tile_pool(name="op", bufs=6) as op,
        ):
            wt = wp.tile([KP, NCH * NM * 128], bf16)     # [(f,c), (j, m, col)]
            wr = w.rearrange("k j m n -> k (j m n)")
            xtiles = {}

            def issue_w(j, m=None):
                if m is None:
                    sl = slice(j * NM * 128, (j + 1) * NM * 128)
                else:
                    sl = slice((j * NM + m) * 128, (j * NM + m + 1) * 128)
                nc.sync.dma_start(out=wt[:, sl], in_=wr[:, sl])

            def issue_x(blk):
                off, n = BOFF[blk], BLOCKS[blk]
                xt = xp.tile([KP, NCH * n], bf16, tag="xt", name=f"xt_{blk}")
                nc.sync.dma_start(out=xt, in_=xsc[:, NCH * off:NCH * (off + n)])
                xtiles[blk] = xt

            def wslice(j, m):
                return wt[:, (j * NM + m) * 128:(j * NM + m + 1) * 128]

            # DMA order: first matmul gates on W(j0)+X0 (~0.25MB); later
            # pieces and x blocks stream in behind it.
            issue_w(0); issue_x(0)
            issue_w(1); issue_x(1)
            issue_w(2); issue_x(2)
            issue_w(3); issue_w(4)

            for blk in range(NBLK):
                off, n = BOFF[blk], BLOCKS[blk]
                if blk + 3 < NBLK:
                    issue_x(blk + 3)
                xt = xtiles.pop(blk)
                pss = [
                    pp.tile([128, n], f32, tag="ps", name=f"ps_{blk}_{m}")
                    for m in range(NM)
                ]
                ot = op.tile([128, NM * n], bf16, tag="ot", name=f"ot_{blk}")

                def evac(m, n=n, pss=pss, ot=ot):
                    nc.vector.tensor_copy(out=ot[:, m * n:(m + 1) * n], in_=pss[m])

                if blk < N_JOUTER:
                    # j-outer: each arriving W(j) piece feeds all 3 M-tiles.
                    for j in range(NCH):
                        for m in range(NM):
                            nc.tensor.matmul(
                                pss[m],
                                wslice(j, m),
                                xt[:, j * n:(j + 1) * n],
                                start=(j == 0),
                                stop=(j == NCH - 1),
                                skip_group_check=True,
                            )
                    for m in range(NM):
                        evac(m)
                else:
                    # m-outer: M-tiles complete one after another, so PSUM
                    # evacuation staggers across the block.
                    for m in range(NM):
                        for j in range(NCH):
                            nc.tensor.matmul(
                                pss[m],
                                wslice(j, m),
                                xt[:, j * n:(j + 1) * n],
                                start=(j == 0),
                                stop=(j == NCH - 1),
                            )
                        evac(m)
                nc.sync.dma_start(
                    out=osc[:, NM * off:NM * (off + n)], in_=ot
                )
    nc.compile()   # bacc passes: split multi-waits (HW allows 1 wait/inst), DCE, reg alloc
    return nc


def kernel(x: np.ndarray, templates: np.ndarray, onset_delays: np.ndarray) -> np.ndarray:
    global LAST_RESULT
    x = np.ascontiguousarray(x, dtype=np.float32)
    templates = np.asarray(templates, dtype=np.float32)
    onset_delays = np.asarray(onset_delays, dtype=np.float32)

    W = _build_weights(templates, onset_delays)
    Xsc = _build_xsc(x)                                   # (NCORES, KP, NCH*CTOT)

    nc = _build_program()
    in_maps = [{"xsc": Xsc[c], "w": W} for c in range(NCORES)]
    res = run_bass_kernel_spmd(nc, in_maps, core_ids=list(range(NCORES)))
    LAST_RESULT = res

    osc = np.stack([r["osc"] for r in res.results], axis=0)   # (NCORES,128,NM*CTOT)
    osc = osc.astype(np.float32)
    O = np.empty((NCORES, 128, NM, CTOT), dtype=np.float32)
    for off, n in zip(BOFF, BLOCKS):
        O[:, :, :, off:off + n] = (
            osc[:, :, NM * off:NM * (off + n)].reshape(NCORES, 128, NM, n)
        )
    o = O.reshape(NCORES, 8, E, NM, BPC, NCOLB)           # c, dd, e, m, b, n
    o = o.transpose(0, 4, 5, 3, 1, 2)                      # c, b, n, m, dd, e
    o = np.ascontiguousarray(o).reshape(B, NCOLB * Q, E)[:, :S, :]
    o = np.ascontiguousarray(o)
    o[:, S - 1, :] = 0.0                                   # reference zero-pads last column
    return o
